# revision 9
# baseline (speedup 1.0000x reference)
"""Trainium2 Bass kernel for nn_CrossChannelAttention.

Reference computation (per batch b, pixel p, with C=128 channels, NUMS=16
groups of HEADS=8 channels, OUT=256):
    fm[g,p]  = relu(sum_h W1[g,h] * x[8g+h, p] + b1[g])          # [16, P]
    feat[(g,d), p] = fm[g,p] * x[d,p]                            # [2048, P]
    out[o,p] = sum_c W2[o,c] * feat[c,p] + b2[o]                 # [256, P]

Strategy: data-parallel over batch B=8 across the 8 NeuronCores (one image
per core, params replicated).  Per core the PE-bound floor is 256 bf16
matmuls [K=128,M=128,N=512] ~= 57us; everything else must hide under it.

v4 (from v2=90.8us, v3=100.4us traces):
  - Two 2048-pixel phases, each holding all 8 PSUM banks (4 pixel blocks x
    2 output-channel halves) while all 16 groups accumulate.
  - Rep broadcasts: one [128,2048] DMA per (g>=4, phase) + [128,1024]
    halves for g=1..3 (earlier availability); g=0 via gpsimd
    partition_broadcast straight from the fm SBUF tile (partition 0 is the
    only legal gpsimd source = row g=0).  ~36 DMA triggers vs v2's 76
    (0.6us of ring-engine time each).
  - ALL phase-A rep tiles are SBUF-resident (full bufs) - v3 lost 12us to
    the rep slot rotation serializing rep delivery behind PE consumption.
  - Phase-B reps prefetch during phase A; phase B runs with zero PE gaps.
  - x loaded in [128,512] quarters so the fm->relu->rep chain starts ~4us
    earlier; fm relus alternate scalar/DVE to halve that chain.
  - fmB matmuls interleave into mains g=0 (banks pb3 via tag rotation
    created before phase-A accumulators); fmB relus on DVE.
  - Emission order rule: every tile write is emitted before its readers
    (trace order defines Tile dependency direction).
Accuracy: bf16 matmuls with fp32 PSUM accumulation; rel err ~4e-3.
"""

import numpy as np
import ml_dtypes

import concourse.bacc as bacc
import concourse.tile as tile
from concourse import mybir
from concourse.bass_utils import run_bass_kernel_spmd

F32 = mybir.dt.float32
BF16 = mybir.dt.bfloat16

B, C, H, W = 8, 128, 64, 64
NUMS, HEADS, OUT = 16, 8, 256
P = H * W          # 4096 pixels per image
PB = 512           # pixel block (one PSUM bank of fp32)
PH = 2048          # phase width (4 pixel blocks; all 8 PSUM banks)
N_CORES = 8

_CACHE = {}


def _build():
    nc = bacc.Bacc("TRN2", target_bir_lowering=False, debug=False,
                   num_devices=N_CORES)

    x_d = nc.dram_tensor("x", [C, P], BF16, kind="ExternalInput")
    w1s_d = nc.dram_tensor("w1s", [C, 256], BF16, kind="ExternalInput")
    w2t_d = nc.dram_tensor("w2t", [C, NUMS * OUT], BF16, kind="ExternalInput")
    b1_d = nc.dram_tensor("b1c", [NUMS, 128], F32, kind="ExternalInput")
    b2_d = nc.dram_tensor("b2c", [C, 128], F32, kind="ExternalInput")
    out_d = nc.dram_tensor("out", [OUT, P], BF16, kind="ExternalOutput")

    relu = mybir.ActivationFunctionType.Relu
    ident = mybir.ActivationFunctionType.Identity
    mult = mybir.AluOpType.mult
    add = mybir.AluOpType.add
    amax = mybir.AluOpType.max

    with tile.TileContext(nc) as tc:
        with (
            tc.tile_pool(name="const", bufs=1) as cpool,
            tc.tile_pool(name="repA", bufs=1) as repAp,
            tc.tile_pool(name="repB", bufs=1) as repBp,
            tc.tile_pool(name="ft", bufs=1) as ftp,
            tc.tile_pool(name="osb", bufs=1) as osbp,
            tc.tile_pool(name="ps", bufs=1, space="PSUM") as ps,
            tc.tile_pool(name="dr", bufs=1, space="DRAM") as drp,
        ):
            # ---- constants / inputs ----
            scratch = cpool.tile([C, C + PB], BF16)
            nc.vector.memset(scratch[:], 0.0)

            w1s_t = cpool.tile([C, 256], BF16)
            b1_t = cpool.tile([NUMS, 128], F32)
            b2_t = cpool.tile([C, 128], F32)
            xA = cpool.tile([C, PH], BF16, name="xA")
            xB = cpool.tile([C, PH], BF16, name="xB")
            w2c = [cpool.tile([C, PH], BF16, name=f"w2c{j}") for j in range(2)]

            # sync ring loads in need-order; xA in quarters so the fm chain
            # starts as early as possible
            nc.sync.dma_start(w1s_t[:], w1s_d[:])
            for q in range(4):
                qx = slice(q * PB, (q + 1) * PB)
                nc.sync.dma_start(xA[:, qx], x_d[:, qx])
            nc.sync.dma_start(w2c[0][:], w2t_d[:, 0:PH])
            nc.sync.dma_start(xB[:, 0:1024], x_d[:, PH:PH + 1024])
            nc.sync.dma_start(xB[:, 1024:PH], x_d[:, PH + 1024:P])
            nc.sync.dma_start(w2c[1][:], w2t_d[:, PH:2 * PH])
            # scalar ring: biases
            nc.scalar.dma_start(b1_t[:], b1_d[:])
            nc.scalar.dma_start(b2_t[:], b2_d[:])

            # ---- PSUM tiles: 8 banks, tag-per-bank, serial reuse ----
            def psum(pb, oc, nm, parts=C):
                return ps.tile([parts, PB], F32, tag=f"pso_{pb}_{oc}",
                               name=nm)

            # first warmup matmul ramps the HAM clock gate from ~7us
            ps_w = psum(0, 0, "ps_warm")
            nc.tensor.matmul(ps_w[:], scratch[:, 0:C], scratch[:, C:C + PB],
                             start=True, stop=True)

            # ---- fm phase A: 4 matmuls [16,512]; relus alternate
            #      scalar (even quarters) / DVE (odd quarters) ----
            fmhA = cpool.tile([NUMS, PH], BF16, name="fmhA")
            fmhB = cpool.tile([NUMS, PH], BF16, name="fmhB")
            fm_drA = drp.tile([NUMS, PH], BF16, name="fm_drA")
            fm_drB = drp.tile([NUMS, PH], BF16, name="fm_drB")
            fm_psA_tags = [(1, 0), (1, 1), (2, 0), (2, 1)]
            for i in range(4):
                qx = slice(i * PB, (i + 1) * PB)
                pf = psum(*fm_psA_tags[i], nm=f"psfmA{i}", parts=NUMS)
                nc.tensor.matmul(pf[:], w1s_t[:, 0:NUMS], xA[:, qx],
                                 start=True, stop=True)
                if i % 2 == 0:
                    nc.scalar.activation(fmhA[:, qx], pf[:], relu,
                                         bias=b1_t[:, 0:1])
                else:
                    nc.vector.tensor_scalar(fmhA[:, qx], pf[:],
                                            b1_t[:, 0:1], 0.0,
                                            op0=add, op1=amax)
                if i == 1:
                    nc.scalar.dma_start(fm_drA[:, 0:1024],
                                        fmhA[0:NUMS, 0:1024])
                if i == 3:
                    nc.scalar.dma_start(fm_drA[:, 1024:PH],
                                        fmhA[0:NUMS, 1024:PH])
            # two more warmups while the rep pipeline fills
            nc.tensor.matmul(ps_w[:], scratch[:, 0:C], scratch[:, C:C + PB],
                             start=True, stop=True)
            nc.tensor.matmul(ps_w[:], scratch[:, 0:C], scratch[:, C:C + PB],
                             start=True, stop=True)
            # phase-B fm psum tiles: created before phase-A main accumulators
            # so the pso_3_* tag rotation orders fmB before mains pb3
            fm_psB_tags = [(3, 0), (3, 1), (3, 0), (3, 1)]
            pfB = [psum(*fm_psB_tags[i], nm=f"psfmB{i}", parts=NUMS)
                   for i in range(4)]

            # g=0 phase-A reps via gpsimd in [128,512] quarters
            rep0q = []
            for i in range(4):
                qx = slice(i * PB, (i + 1) * PB)
                r = repAp.tile([C, PB], BF16, name=f"rep0q{i}")
                nc.gpsimd.partition_broadcast(r[:], fmhA[0:1, qx])
                rep0q.append(r)

            # ---- rep DMA broadcasts, phase A (all emitted before their ft
            #      consumers; all tiles fully resident - no slot rotation) ----
            # g=1..3 in [128,1024] halves: h0 needs only fm_drA[:,0:1024]
            repAh = {}
            for g in (1, 2, 3):
                for h in range(2):
                    repAh[(g, h)] = repAp.tile([C, 1024], BF16,
                                               name=f"rep{g}h{h}")

            def emit_repAh(g, h):
                hx = slice(h * 1024, (h + 1) * 1024)
                eng = nc.sync if g == 1 else nc.scalar
                eng.dma_start(repAh[(g, h)][:],
                              fm_drA[g:g + 1, hx].broadcast_to((C, 1024)))

            for h in range(2):
                for g in (1, 2, 3):
                    emit_repAh(g, h)
            repA = {}
            for g in range(4, NUMS):
                repA[g] = repAp.tile([C, PH], BF16, tag="repfA", bufs=12,
                                     name=f"repA{g}")
            for g in range(4, NUMS):
                eng = nc.sync if (g % 2 == 0) else nc.scalar
                eng.dma_start(repA[g][:],
                              fm_drA[g:g + 1, :].broadcast_to((C, PH)))

            # ---- feat producers (DVE): g=0 quarters, g=1 halves ----
            ftA = {}
            for i in range(4):
                qx = slice(i * PB, (i + 1) * PB)
                f = ftp.tile([C, PB], BF16, name=f"ft0q{i}")
                nc.vector.tensor_tensor(f[:], xA[:, qx], rep0q[i][:], op=mult)
                ftA[(0, i)] = f
            for h in range(2):
                hx = slice(h * 1024, (h + 1) * 1024)
                f = ftp.tile([C, 1024], BF16, name=f"ft1h{h}")
                nc.vector.tensor_tensor(f[:], xA[:, hx], repAh[(1, h)][:],
                                        op=mult)
                ftA[(1, h)] = f

            def rhsA(g, pb):
                if g == 0:
                    return ftA[(0, pb)][:]
                if g in (1, 2, 3):
                    h, r = divmod(pb, 2)
                    return ftA[(g, h)][:, r * PB:(r + 1) * PB]
                return ftA[g][:, pb * PB:(pb + 1) * PB]

            def w2blk(g, oc):
                j, r = divmod(g, 8)
                cx = slice((r * 2 + oc) * C, (r * 2 + oc + 1) * C)
                return w2c[j][:, cx]

            # ---- main matmuls phase A, fmB interleaved into g=0 ----
            psoA = {(pb, oc): psum(pb, oc, f"psoA{pb}_{oc}")
                    for pb in range(4) for oc in range(2)}

            def emit_fmB(i):
                qx = slice(i * PB, (i + 1) * PB)
                nc.tensor.matmul(pfB[i][:], w1s_t[:, 0:NUMS], xB[:, qx],
                                 start=True, stop=True)
                nc.vector.tensor_scalar(fmhB[:, qx], pfB[i][:],
                                        b1_t[:, 0:1], 0.0,
                                        op0=add, op1=amax)

            for g in range(NUMS):
                for pb in range(4):
                    for oc in range(2):
                        nc.tensor.matmul(psoA[(pb, oc)][:], w2blk(g, oc),
                                         rhsA(g, pb),
                                         start=(g == 0), stop=(g == NUMS - 1))
                    if g == 0 and pb == 0:
                        emit_fmB(0)
                        emit_fmB(1)
                    if g == 0 and pb == 1:
                        emit_fmB(2)
                        emit_fmB(3)
                if g == 0:
                    # DVE stream: g=2,3 ft halves right after the fmB relus
                    for gg in (2, 3):
                        for h in range(2):
                            hx = slice(h * 1024, (h + 1) * 1024)
                            f = ftp.tile([C, 1024], BF16, name=f"ft{gg}h{h}")
                            nc.vector.tensor_tensor(f[:], xA[:, hx],
                                                    repAh[(gg, h)][:],
                                                    op=mult)
                            ftA[(gg, h)] = f
                    # phase-B g=0 rep via gpsimd; fm_drB on the scalar ring
                    rep0B = repBp.tile([C, PH], BF16, name="rep0B")
                    nc.gpsimd.partition_broadcast(rep0B[:], fmhB[0:1, :])
                    nc.scalar.dma_start(fm_drB[:, 0:1024],
                                        fmhB[0:NUMS, 0:1024])
                    nc.scalar.dma_start(fm_drB[:, 1024:PH],
                                        fmhB[0:NUMS, 1024:PH])
                    # remaining phase-A feat producers
                    for gg in range(4, NUMS):
                        f = ftp.tile([C, PH], BF16, tag="ftfA", bufs=5,
                                     name=f"ftA{gg}")
                        nc.vector.tensor_tensor(f[:], xA[:], repA[gg][:],
                                                op=mult)
                        ftA[gg] = f
                if g == 1:
                    # phase-B rep prefetch (ring-ordered behind phase A's)
                    repB = {}
                    for gb in range(1, NUMS):
                        repB[gb] = repBp.tile([C, PH], BF16, tag="repfB",
                                              bufs=13, name=f"repB{gb}")
                    for gb in range(1, NUMS):
                        eng = nc.sync if (gb % 2 == 0) else nc.scalar
                        eng.dma_start(
                            repB[gb][:],
                            fm_drB[gb:gb + 1, :].broadcast_to((C, PH)))

            # ---- feat producers phase B (head of DVE tail) ----
            ftB = {}
            f = ftp.tile([C, PH], BF16, name="ftB0")
            nc.vector.tensor_tensor(f[:], xB[:], rep0B[:], op=mult)
            ftB[0] = f
            for g in range(1, 3):
                f = ftp.tile([C, PH], BF16, tag="ftfB", bufs=3,
                             name=f"ftB{g}")
                nc.vector.tensor_tensor(f[:], xB[:], repB[g][:], op=mult)
                ftB[g] = f

            # ---- drains + stores phase A (chase the PE bank order) ----
            osbA = {(pp, oc): osbp.tile([C, 1024], BF16, tag="osb", bufs=4,
                                        name=f"osbA{pp}_{oc}")
                    for pp in range(2) for oc in range(2)}
            for pb in range(4):
                pp, r = divmod(pb, 2)
                sx = slice(r * PB, (r + 1) * PB)
                nc.scalar.activation(osbA[(pp, 0)][:, sx],
                                     psoA[(pb, 0)][:], ident,
                                     bias=b2_t[:, 0:1])
                nc.vector.tensor_scalar_add(osbA[(pp, 1)][:, sx],
                                            psoA[(pb, 1)][:], b2_t[:, 1:2])
            for pp in range(2):
                px = slice(pp * 1024, (pp + 1) * 1024)
                nc.sync.dma_start(out_d[0:C, px], osbA[(pp, 0)][:])
                nc.scalar.dma_start(out_d[C:OUT, px], osbA[(pp, 1)][:])

            # remaining phase-B feat producers
            for g in range(3, NUMS):
                f = ftp.tile([C, PH], BF16, tag="ftfB", bufs=3,
                             name=f"ftB{g}")
                nc.vector.tensor_tensor(f[:], xB[:], repB[g][:], op=mult)
                ftB[g] = f

            # ---- main matmuls phase B ----
            psoB = {(pb, oc): psum(pb, oc, f"psoB{pb}_{oc}")
                    for pb in range(4) for oc in range(2)}
            for g in range(NUMS):
                for pb in range(4):
                    rhs = ftB[g][:, pb * PB:(pb + 1) * PB]
                    for oc in range(2):
                        nc.tensor.matmul(psoB[(pb, oc)][:], w2blk(g, oc),
                                         rhs,
                                         start=(g == 0), stop=(g == NUMS - 1))

            # ---- drains + stores phase B ----
            osbB = {(pp, oc): osbp.tile([C, 1024], BF16, tag="osb", bufs=4,
                                        name=f"osbB{pp}_{oc}")
                    for pp in range(2) for oc in range(2)}
            for pb in range(4):
                pp, r = divmod(pb, 2)
                sx = slice(r * PB, (r + 1) * PB)
                nc.scalar.activation(osbB[(pp, 0)][:, sx],
                                     psoB[(pb, 0)][:], ident,
                                     bias=b2_t[:, 0:1])
                nc.vector.tensor_scalar_add(osbB[(pp, 1)][:, sx],
                                            psoB[(pb, 1)][:], b2_t[:, 1:2])
            for pp in range(2):
                px = slice(PH + pp * 1024, PH + (pp + 1) * 1024)
                nc.sync.dma_start(out_d[0:C, px], osbB[(pp, 0)][:])
                nc.scalar.dma_start(out_d[C:OUT, px], osbB[(pp, 1)][:])

    nc.compile()
    return nc


def _prep_params(W1, b1, W2, b2):
    bf = ml_dtypes.bfloat16
    w1s = np.zeros((C, 256), dtype=bf)
    for g in range(NUMS):
        w1s[g * HEADS:(g + 1) * HEADS, g] = W1[g].astype(bf)
    w2t = (
        np.asarray(W2, dtype=np.float32)
        .reshape(2, C, NUMS, C)          # [oc, m, g, k]
        .transpose(3, 2, 0, 1)           # [k, g, oc, m]
        .reshape(C, NUMS * OUT)
        .astype(bf)
    )
    b1c = np.zeros((NUMS, 128), dtype=np.float32)
    b1c[:, 0] = np.asarray(b1, dtype=np.float32)
    b2c = np.zeros((C, 128), dtype=np.float32)
    b2c[:, 0:2] = np.asarray(b2, dtype=np.float32).reshape(2, C).T
    return w1s, w2t, b1c, b2c


def kernel(x, W1, b1, W2, b2, _trace=False, _trace_kwargs=None):
    if "nc" not in _CACHE:
        _CACHE["nc"] = _build()
    nc = _CACHE["nc"]

    w1s, w2t, b1c, b2c = _prep_params(W1, b1, W2, b2)
    xs = np.ascontiguousarray(
        np.asarray(x, dtype=np.float32).reshape(B, C, P).astype(ml_dtypes.bfloat16))
    in_maps = [
        {"x": xs[b_], "w1s": w1s, "w2t": w2t, "b1c": b1c, "b2c": b2c}
        for b_ in range(N_CORES)
    ]
    kwargs = {}
    if _trace:
        kwargs["trace"] = True
        kwargs.update(_trace_kwargs or {})
    res = run_bass_kernel_spmd(nc, in_maps, core_ids=list(range(N_CORES)),
                               **kwargs)
    out = np.stack([np.asarray(res.results[b_]["out"], dtype=np.float32)
                    for b_ in range(N_CORES)])
    out = out.reshape(B, OUT, H, W)
    if _trace:
        _CACHE["last_result"] = res
    return out


# revision 14
# speedup vs baseline: 1.0379x; 1.0379x over previous
"""Trainium2 Bass kernel for nn_CrossChannelAttention.

Reference computation (per batch b, pixel p, with C=128 channels, NUMS=16
groups of HEADS=8 channels, OUT=256):
    fm[g,p]  = relu(sum_h W1[g,h] * x[8g+h, p] + b1[g])          # [16, P]
    feat[(g,d), p] = fm[g,p] * x[d,p]                            # [2048, P]
    out[o,p] = sum_c W2[o,c] * feat[c,p] + b2[o]                 # [256, P]

Strategy: data-parallel over batch B=8 across the 8 NeuronCores (one image
per core, params replicated).  Per core the PE-bound floor is 256 bf16
matmuls [K=128,M=128,N=512] ~= 57us; everything else must hide under it.

v5 (from v2=90.8, v3=100.4, v4=102.5 traces):
  - The two HWDGE rings TOGETHER sustain only ~280GB/s of broadcast SBUF
    writes (shared DMA-engine pool), exactly the rep-broadcast demand of a
    full-speed phase.  So gpsimd partition_broadcast carries 7 of the 32
    rep units (g0/g7/g11 of phase A, g0/g5/g9/g13 of phase B; g>0 sources
    via tiny p0-row copies from the fm DRAM mirror, since gpsimd can only
    read partition 0).
  - Every rep tile is fully SBUF-resident (bufs = tile count): v3/v4 lost
    10+us to slot-rotation serializing rep delivery behind PE consumption,
    and to rotating triggers blocking ring queues ahead of phase-B drains.
  - Loads split across rings (sync: w1s + xA quarters + w2c1; scalar:
    biases + w2c0 + xB halves) so the fm chain starts ~8us.
  - fm relus: A0/A1/A3 + all fmB on DVE (tensor_scalar add+max), A2 on
    scalar; DVE stream hand-ordered so every PE dependency lands just
    ahead of its matmul.
  - Two 2048-px phases x 8 PSUM banks; fmB matmuls interleave into mains
    g=0 (pso_3_* tag rotation: pfB created before psoA); phase-B reps
    prefetch behind phase A's in ring order; drains chase the PE bank
    order (scalar=oc0, vector=oc1); stores [128,1024] split across rings.
Accuracy: bf16 matmuls with fp32 PSUM accumulation; rel err ~4e-3.
"""

import numpy as np
import ml_dtypes

import concourse.bacc as bacc
import concourse.tile as tile
from concourse import mybir
from concourse.bass_utils import run_bass_kernel_spmd

F32 = mybir.dt.float32
BF16 = mybir.dt.bfloat16

B, C, H, W = 8, 128, 64, 64
NUMS, HEADS, OUT = 16, 8, 256
P = H * W          # 4096 pixels per image
PB = 512           # pixel block (one PSUM bank of fp32)
PH = 2048          # phase width (4 pixel blocks; all 8 PSUM banks)
N_CORES = 8

GPS_A = (7, 11)        # phase-A units on gpsimd (plus g=0)
GPS_B = (5, 9, 13)     # phase-B units on gpsimd (plus g=0)

_CACHE = {}


def _build():
    nc = bacc.Bacc("TRN2", target_bir_lowering=False, debug=False,
                   num_devices=N_CORES)

    x_d = nc.dram_tensor("x", [C, P], BF16, kind="ExternalInput")
    w1s_d = nc.dram_tensor("w1s", [C, 256], BF16, kind="ExternalInput")
    w2t_d = nc.dram_tensor("w2t", [C, NUMS * OUT], BF16, kind="ExternalInput")
    b1_d = nc.dram_tensor("b1c", [NUMS, 128], F32, kind="ExternalInput")
    b2_d = nc.dram_tensor("b2c", [C, 128], F32, kind="ExternalInput")
    out_d = nc.dram_tensor("out", [OUT, P], BF16, kind="ExternalOutput")

    relu = mybir.ActivationFunctionType.Relu
    ident = mybir.ActivationFunctionType.Identity
    mult = mybir.AluOpType.mult
    add = mybir.AluOpType.add
    amax = mybir.AluOpType.max

    def vrelu(out_ap, in_ap, bias_ap):
        nc.vector.tensor_scalar(out_ap, in_ap, bias_ap, 0.0,
                                op0=add, op1=amax)

    with tile.TileContext(nc) as tc:
        with (
            tc.tile_pool(name="const", bufs=1) as cpool,
            tc.tile_pool(name="repA", bufs=1) as repAp,
            tc.tile_pool(name="repB", bufs=1) as repBp,
            tc.tile_pool(name="ft", bufs=1) as ftp,
            tc.tile_pool(name="osb", bufs=1) as osbp,
            tc.tile_pool(name="ps", bufs=1, space="PSUM") as ps,
            tc.tile_pool(name="dr", bufs=1, space="DRAM") as drp,
        ):
            # ---- constants / inputs ----
            scratch = cpool.tile([C, C + PB], BF16)
            nc.vector.memset(scratch[:], 0.0)

            w1s_t = cpool.tile([C, 256], BF16)
            b1_t = cpool.tile([NUMS, 128], F32)
            b2_t = cpool.tile([C, 128], F32)
            xA = cpool.tile([C, PH], BF16, name="xA")
            xB = cpool.tile([C, PH], BF16, name="xB")
            w2c = [cpool.tile([C, PH], BF16, name=f"w2c{j}") for j in range(2)]

            # sync: w1s + xA quarters (fm chain) + w2c1; scalar: biases +
            # w2c0 (mains g0) + xB halves (fmB)
            nc.sync.dma_start(w1s_t[:], w1s_d[:])
            for q in range(4):
                qx = slice(q * PB, (q + 1) * PB)
                nc.sync.dma_start(xA[:, qx], x_d[:, qx])
            nc.sync.dma_start(w2c[1][:], w2t_d[:, PH:2 * PH])
            nc.scalar.dma_start(b1_t[:], b1_d[:])
            nc.scalar.dma_start(b2_t[:], b2_d[:])
            nc.scalar.dma_start(w2c[0][:], w2t_d[:, 0:PH])
            nc.scalar.dma_start(xB[:, 0:1024], x_d[:, PH:PH + 1024])
            nc.scalar.dma_start(xB[:, 1024:PH], x_d[:, PH + 1024:P])

            # ---- PSUM tiles: 8 banks, tag-per-bank, serial reuse ----
            def psum(pb, oc, nm, parts=C):
                return ps.tile([parts, PB], F32, tag=f"pso_{pb}_{oc}",
                               name=nm)

            # warmup matmul ramps the HAM clock gate from ~7.4us
            ps_w = psum(0, 0, "ps_warm")

            def warm():
                nc.tensor.matmul(ps_w[:], scratch[:, 0:C],
                                 scratch[:, C:C + PB], start=True, stop=True)

            warm()

            # ---- fm phase A: relus A0/A1/A3 on DVE, A2 on scalar ----
            fmhA = cpool.tile([NUMS, PH], BF16, name="fmhA")
            fmhB = cpool.tile([NUMS, PH], BF16, name="fmhB")
            fm_drA = drp.tile([NUMS, PH], BF16, name="fm_drA")
            fm_drB = drp.tile([NUMS, PH], BF16, name="fm_drB")
            fm_psA_tags = [(1, 0), (1, 1), (2, 0), (2, 1)]
            for i in range(4):
                qx = slice(i * PB, (i + 1) * PB)
                pf = psum(*fm_psA_tags[i], nm=f"psfmA{i}", parts=NUMS)
                nc.tensor.matmul(pf[:], w1s_t[:, 0:NUMS], xA[:, qx],
                                 start=True, stop=True)
                if i == 2:
                    nc.scalar.activation(fmhA[:, qx], pf[:], relu,
                                         bias=b1_t[:, 0:1])
                else:
                    vrelu(fmhA[:, qx], pf[:], b1_t[:, 0:1])
            # fm DRAM mirror halves (scalar ring), as soon as each is ready
            nc.scalar.dma_start(fm_drA[:, 0:1024], fmhA[0:NUMS, 0:1024])
            nc.scalar.dma_start(fm_drA[:, 1024:PH], fmhA[0:NUMS, 1024:PH])
            warm()
            warm()
            # phase-B fm psum tiles: created before phase-A main accumulators
            # so the pso_3_* tag rotation orders fmB before mains pb3
            fm_psB_tags = [(3, 0), (3, 1), (3, 0), (3, 1)]
            pfB = [psum(*fm_psB_tags[i], nm=f"psfmB{i}", parts=NUMS)
                   for i in range(4)]

            # g=0 phase-A reps via gpsimd in [128,512] quarters
            rep0q = []
            for i in range(4):
                qx = slice(i * PB, (i + 1) * PB)
                r = repAp.tile([C, PB], BF16, name=f"rep0q{i}")
                nc.gpsimd.partition_broadcast(r[:], fmhA[0:1, qx])
                rep0q.append(r)
            # p0-row copies for gpsimd units: [1,1024] halves rotating
            # through 2 column slots (gpsimd can only read partition 0)
            p0rows = {}

            def p0row(ph, g, h):
                t = cpool.tile([1, 1024], BF16, tag="p0row", bufs=2,
                               name=f"p0r{ph}{g}h{h}")
                p0rows[(ph, g, h)] = t
                return t

            # ---- rep DMA broadcasts, phase A (fully resident tiles) ----
            repAh = {}
            for g in (1, 2, 3):
                for h in range(2):
                    repAh[(g, h)] = repAp.tile([C, 1024], BF16,
                                               name=f"rep{g}h{h}")
            dmaA = [g for g in range(4, NUMS) if g not in GPS_A]
            repA = {}
            for g in dmaA:
                repA[g] = repAp.tile([C, PH], BF16, tag="repfA",
                                     bufs=8, name=f"repA{g}")
            # sync: g1 halves, p0-rows for gpsimd, then even fulls;
            # scalar: g2/g3 halves then odd fulls (in consumption order)
            for h in range(2):
                hx = slice(h * 1024, (h + 1) * 1024)
                nc.sync.dma_start(repAh[(1, h)][:],
                                  fm_drA[1:2, hx].broadcast_to((C, 1024)))
            for g in GPS_A:
                for h in range(2):
                    hx = slice(h * 1024, (h + 1) * 1024)
                    nc.sync.dma_start(p0row("A", g, h)[:],
                                      fm_drA[g:g + 1, hx])
            repAg = {}
            for g in GPS_A:
                r = repAp.tile([C, PH], BF16, name=f"repAg{g}")
                for h in range(2):
                    hx = slice(h * 1024, (h + 1) * 1024)
                    nc.gpsimd.partition_broadcast(r[:, hx],
                                                  p0rows[("A", g, h)][:])
                repAg[g] = r
            for g in (2, 3):
                for h in range(2):
                    hx = slice(h * 1024, (h + 1) * 1024)
                    nc.scalar.dma_start(
                        repAh[(g, h)][:],
                        fm_drA[g:g + 1, hx].broadcast_to((C, 1024)))
            # even fulls on sync now; odd fulls go on scalar inside the
            # main loop (after the tiny fm_drB triggers)
            for g in dmaA:
                if g % 2 == 0:
                    nc.sync.dma_start(
                        repA[g][:], fm_drA[g:g + 1, :].broadcast_to((C, PH)))

            # ---- feat producers (DVE) for g=0/g=1 + main loop ----
            ftA = {}
            for i in (0, 1):
                qx = slice(i * PB, (i + 1) * PB)
                f = ftp.tile([C, PB], BF16, name=f"ft0q{i}")
                nc.vector.tensor_tensor(f[:], xA[:, qx], rep0q[i][:], op=mult)
                ftA[(0, i)] = f

            def rhsA(g, pb):
                if g == 0:
                    return ftA[(0, pb)][:]
                if g in (1, 2, 3):
                    h, r = divmod(pb, 2)
                    return ftA[(g, h)][:, r * PB:(r + 1) * PB]
                return ftA[g][:, pb * PB:(pb + 1) * PB]

            def w2blk(g, oc):
                j, r = divmod(g, 8)
                cx = slice((r * 2 + oc) * C, (r * 2 + oc + 1) * C)
                return w2c[j][:, cx]

            psoA = {(pb, oc): psum(pb, oc, f"psoA{pb}_{oc}")
                    for pb in range(4) for oc in range(2)}

            def emit_fmB(i):
                qx = slice(i * PB, (i + 1) * PB)
                nc.tensor.matmul(pfB[i][:], w1s_t[:, 0:NUMS], xB[:, qx],
                                 start=True, stop=True)

            def emit_fmB_relu(i):
                qx = slice(i * PB, (i + 1) * PB)
                vrelu(fmhB[:, qx], pfB[i][:], b1_t[:, 0:1])

            for g in range(NUMS):
                for pb in range(4):
                    for oc in range(2):
                        nc.tensor.matmul(psoA[(pb, oc)][:], w2blk(g, oc),
                                         rhsA(g, pb),
                                         start=(g == 0), stop=(g == NUMS - 1))
                    if g == 0 and pb == 0:
                        # PE: fmB k0/k1; DVE: reluB0, reluB1, then ft0q2/q3
                        emit_fmB(0)
                        emit_fmB(1)
                        emit_fmB_relu(0)
                        emit_fmB_relu(1)
                        for i in (2, 3):
                            qx = slice(i * PB, (i + 1) * PB)
                            f = ftp.tile([C, PB], BF16, name=f"ft0q{i}")
                            nc.vector.tensor_tensor(f[:], xA[:, qx],
                                                    rep0q[i][:], op=mult)
                            ftA[(0, i)] = f
                    if g == 0 and pb == 1:
                        emit_fmB(2)
                        emit_fmB(3)
                        emit_fmB_relu(2)
                        emit_fmB_relu(3)
                if g == 0:
                    # DVE: g1..g3 ft halves next
                    for gg in (1, 2, 3):
                        for h in range(2):
                            hx = slice(h * 1024, (h + 1) * 1024)
                            f = ftp.tile([C, 1024], BF16, name=f"ft{gg}h{h}")
                            nc.vector.tensor_tensor(f[:], xA[:, hx],
                                                    repAh[(gg, h)][:],
                                                    op=mult)
                            ftA[(gg, h)] = f
                    # fm_drB mirror, then phase-A odd fulls (scalar) so the
                    # ring starts phase-B prep without blocking drains later
                    nc.scalar.dma_start(fm_drB[:, 0:1024],
                                        fmhB[0:NUMS, 0:1024])
                    nc.scalar.dma_start(fm_drB[:, 1024:PH],
                                        fmhB[0:NUMS, 1024:PH])
                    rep0B = repBp.tile([C, PH], BF16, name="rep0B")
                    nc.gpsimd.partition_broadcast(rep0B[:], fmhB[0:1, :])
                    repBg = {}
                    for gb in GPS_B:
                        r = repBp.tile([C, PH], BF16, name=f"repBg{gb}")
                        for h in range(2):
                            hx = slice(h * 1024, (h + 1) * 1024)
                            nc.sync.dma_start(p0row("B", gb, h)[:],
                                              fm_drB[gb:gb + 1, hx])
                            nc.gpsimd.partition_broadcast(
                                r[:, hx], p0rows[("B", gb, h)][:])
                        repBg[gb] = r
                    # phase-A odd fulls on the scalar ring
                    for gg in dmaA:
                        if gg % 2 == 1:
                            nc.scalar.dma_start(
                                repA[gg][:],
                                fm_drA[gg:gg + 1, :].broadcast_to((C, PH)))
                    # remaining phase-A feat producers
                    for gg in range(4, NUMS):
                        src = repAg[gg] if gg in GPS_A else repA[gg]
                        f = ftp.tile([C, PH], BF16, tag="ftfA", bufs=4,
                                     name=f"ftA{gg}")
                        nc.vector.tensor_tensor(f[:], xA[:], src[:], op=mult)
                        ftA[gg] = f
                if g == 1:
                    # phase-B rep prefetch: fully-resident tiles, ring-
                    # ordered behind phase A's transfers
                    dmaB = [gb for gb in range(1, NUMS) if gb not in GPS_B]
                    repB = {}
                    for gb in dmaB:
                        repB[gb] = repBp.tile([C, PH], BF16, tag="repfB",
                                              bufs=len(dmaB),
                                              name=f"repB{gb}")
                    for gb in dmaB:
                        eng = nc.sync if (gb % 2 == 0) else nc.scalar
                        eng.dma_start(
                            repB[gb][:],
                            fm_drB[gb:gb + 1, :].broadcast_to((C, PH)))
                    for gb in GPS_B:
                        repB[gb] = repBg[gb]

            # ---- feat producers phase B (head of DVE tail) ----
            ftB = {}
            f = ftp.tile([C, PH], BF16, name="ftB0")
            nc.vector.tensor_tensor(f[:], xB[:], rep0B[:], op=mult)
            ftB[0] = f
            for g in range(1, 3):
                f = ftp.tile([C, PH], BF16, tag="ftfB", bufs=2,
                             name=f"ftB{g}")
                nc.vector.tensor_tensor(f[:], xB[:], repB[g][:], op=mult)
                ftB[g] = f

            # ---- drains + stores phase A (chase the PE bank order) ----
            osbA = {(pp, oc): osbp.tile([C, 1024], BF16, tag="osb", bufs=4,
                                        name=f"osbA{pp}_{oc}")
                    for pp in range(2) for oc in range(2)}
            for pb in range(4):
                pp, r = divmod(pb, 2)
                sx = slice(r * PB, (r + 1) * PB)
                nc.scalar.activation(osbA[(pp, 0)][:, sx],
                                     psoA[(pb, 0)][:], ident,
                                     bias=b2_t[:, 0:1])
                nc.vector.tensor_scalar_add(osbA[(pp, 1)][:, sx],
                                            psoA[(pb, 1)][:], b2_t[:, 1:2])
            for pp in range(2):
                px = slice(pp * 1024, (pp + 1) * 1024)
                nc.sync.dma_start(out_d[0:C, px], osbA[(pp, 0)][:])
                nc.scalar.dma_start(out_d[C:OUT, px], osbA[(pp, 1)][:])

            # remaining phase-B feat producers
            for g in range(3, NUMS):
                f = ftp.tile([C, PH], BF16, tag="ftfB", bufs=2,
                             name=f"ftB{g}")
                nc.vector.tensor_tensor(f[:], xB[:], repB[g][:], op=mult)
                ftB[g] = f

            # ---- main matmuls phase B ----
            psoB = {(pb, oc): psum(pb, oc, f"psoB{pb}_{oc}")
                    for pb in range(4) for oc in range(2)}
            for g in range(NUMS):
                for pb in range(4):
                    rhs = ftB[g][:, pb * PB:(pb + 1) * PB]
                    for oc in range(2):
                        nc.tensor.matmul(psoB[(pb, oc)][:], w2blk(g, oc),
                                         rhs,
                                         start=(g == 0), stop=(g == NUMS - 1))

            # ---- drains + stores phase B ----
            osbB = {(pp, oc): osbp.tile([C, 1024], BF16, tag="osb", bufs=4,
                                        name=f"osbB{pp}_{oc}")
                    for pp in range(2) for oc in range(2)}
            for pb in range(4):
                pp, r = divmod(pb, 2)
                sx = slice(r * PB, (r + 1) * PB)
                nc.scalar.activation(osbB[(pp, 0)][:, sx],
                                     psoB[(pb, 0)][:], ident,
                                     bias=b2_t[:, 0:1])
                nc.vector.tensor_scalar_add(osbB[(pp, 1)][:, sx],
                                            psoB[(pb, 1)][:], b2_t[:, 1:2])
            for pp in range(2):
                px = slice(PH + pp * 1024, PH + (pp + 1) * 1024)
                nc.sync.dma_start(out_d[0:C, px], osbB[(pp, 0)][:])
                nc.scalar.dma_start(out_d[C:OUT, px], osbB[(pp, 1)][:])

    nc.compile()
    return nc


def _prep_params(W1, b1, W2, b2):
    bf = ml_dtypes.bfloat16
    w1s = np.zeros((C, 256), dtype=bf)
    for g in range(NUMS):
        w1s[g * HEADS:(g + 1) * HEADS, g] = W1[g].astype(bf)
    w2t = (
        np.asarray(W2, dtype=np.float32)
        .reshape(2, C, NUMS, C)          # [oc, m, g, k]
        .transpose(3, 2, 0, 1)           # [k, g, oc, m]
        .reshape(C, NUMS * OUT)
        .astype(bf)
    )
    b1c = np.zeros((NUMS, 128), dtype=np.float32)
    b1c[:, 0] = np.asarray(b1, dtype=np.float32)
    b2c = np.zeros((C, 128), dtype=np.float32)
    b2c[:, 0:2] = np.asarray(b2, dtype=np.float32).reshape(2, C).T
    return w1s, w2t, b1c, b2c


def kernel(x, W1, b1, W2, b2, _trace=False, _trace_kwargs=None):
    if "nc" not in _CACHE:
        _CACHE["nc"] = _build()
    nc = _CACHE["nc"]

    w1s, w2t, b1c, b2c = _prep_params(W1, b1, W2, b2)
    xs = np.ascontiguousarray(
        np.asarray(x, dtype=np.float32).reshape(B, C, P).astype(ml_dtypes.bfloat16))
    in_maps = [
        {"x": xs[b_], "w1s": w1s, "w2t": w2t, "b1c": b1c, "b2c": b2c}
        for b_ in range(N_CORES)
    ]
    kwargs = {}
    if _trace:
        kwargs["trace"] = True
        kwargs.update(_trace_kwargs or {})
    res = run_bass_kernel_spmd(nc, in_maps, core_ids=list(range(N_CORES)),
                               **kwargs)
    out = np.stack([np.asarray(res.results[b_]["out"], dtype=np.float32)
                    for b_ in range(N_CORES)])
    out = out.reshape(B, OUT, H, W)
    if _trace:
        _CACHE["last_result"] = res
    return out
